# Initial kernel scaffold
#
"""Causal self-attention (B=4, T=2048, D=1024, H=16) on 8 Trainium2 cores.

Sharding: core c handles batch b = c // 2 and head-half = c % 2 (8 of the 16
heads). Zero cross-core communication: each core computes q/k/v projections
for its 8 heads, causal flash-style attention, and a partial output
projection against its half of w_o. The host sums the two partial
projections per batch.

Layouts (chosen so every matmul operand is a direct slice, no on-device
transposes):
  xT    (1024, 2048)  x[b].T            rhs of q/k (d on partitions), lhsT of v
  wqT   (1024, 512)   (0.125*w_q[rows]).T  (scale folded in, exact pow2)
  wkT   (1024, 512)   w_k[rows].T
  wvT   (1024, 512)   w_v[rows].T
  woT   (512, 1024)   w_o[:, cols].T
  poutT (1024, 2048)  partial (x @ w_o.T contribution).T

Attention math per head (dh=64): scores are computed TRANSPOSED
(k on partitions, q on free dim) so that softmax(score) tiles feed the
P@V matmul directly as the moving operand. Softmax uses no max-subtraction
(scores are O(5), fp32 exp is safe); the denominator is produced by an
extra all-ones column appended to v (M=65 in the P@V matmul); the
normalization multiplies the 64 output rows by 1/sums, with sums broadcast
across partitions via a K=1 ones matmul and inverted by
reciprocal_approx_fast (sums >= 1 always: the k=q diagonal term is
exp(|q|^2/8) >= 1).

PRECISION:
  "f32r" - all matmuls in float32r (TF32-like, ~2.8e-4 absmax error)
  "lp"   - q/k/x/w in fp16, softmax weights/v in bf16 (exp can reach e^30,
           beyond fp16 range), PSUM always fp32
"""
import sys

if "/opt/trn_rl_repo" not in sys.path:
    sys.path.insert(0, "/opt/trn_rl_repo")

import numpy as np

B, T, D, H = 4, 2048, 1024, 16
P, TQ = 128, 512
ND = D // P          # 8  d-slices (contraction tiles for projections)
NHP = 4              # head-pairs per core (8 heads)
NQB = T // TQ        # 4  q blocks
NKB = T // P         # 16 k tiles

PRECISION = "f32r"

_COMPILED = {}


def _build(precision):
    import concourse.bacc as bacc
    import concourse.tile as tile
    from concourse import mybir
    from contextlib import ExitStack

    F32 = mybir.dt.float32
    F32R = mybir.dt.float32r
    AF = mybir.ActivationFunctionType

    lp = precision == "lp"
    DT_IN = mybir.dt.float16 if lp else F32   # dram input dtype
    DT_X = mybir.dt.float16 if lp else F32R   # x / wq / wk / wv / wo / q / k / ao
    DT_P = mybir.dt.bfloat16 if lp else F32R  # softmax weights and v

    def dma_in(dst, src_ap):
        # f32r tiles are fed by bitcasting the f32 dram source; fp16 matches
        nc.sync.dma_start(dst, src_ap.bitcast(DT_X) if not lp else src_ap)

    nc = bacc.Bacc("TRN2", target_bir_lowering=False, debug=False, num_devices=8)

    xT = nc.dram_tensor("xT", [D, T], DT_IN, kind="ExternalInput")
    wqT = nc.dram_tensor("wqT", [D, 512], DT_IN, kind="ExternalInput")
    wkT = nc.dram_tensor("wkT", [D, 512], DT_IN, kind="ExternalInput")
    wvT = nc.dram_tensor("wvT", [D, 512], DT_IN, kind="ExternalInput")
    woT = nc.dram_tensor("woT", [512, D], DT_IN, kind="ExternalInput")
    pout = nc.dram_tensor("poutT", [D, T], F32, kind="ExternalOutput")

    with tile.TileContext(nc) as tc:
        with ExitStack() as ctx:
            q_pool = ctx.enter_context(tc.tile_pool(name="q", bufs=NHP))
            k_pool = ctx.enter_context(tc.tile_pool(name="k", bufs=NHP))
            v_pool = ctx.enter_context(tc.tile_pool(name="v", bufs=NKB))
            qT = [q_pool.tile([P, T], DT_X, tag="qT", name="qT") for _ in range(NHP)]
            kT = [k_pool.tile([P, T], DT_X, tag="kT", name="kT") for _ in range(NHP)]
            # v, row-major (k-position on partitions), 65th column = 1.0
            vA = [v_pool.tile([P, 8, 65], DT_P, tag="vA", name="vA") for _ in range(NKB)]

            # ---------------- q/k/v projections ----------------
            with tc.tile_pool(name="xt", bufs=ND) as xt_pool, \
                 tc.tile_pool(name="w", bufs=17) as w_pool, \
                 tc.tile_pool(name="mmps", bufs=6, space="PSUM") as mm_psum:
                xt = []
                for ds in range(ND):
                    t = xt_pool.tile([P, T], DT_X, tag="xt", name="xt")
                    dma_in(t, xT[ds * P:(ds + 1) * P, :])
                    xt.append(t)

                # v first (row-major: t on partitions) so attention for the
                # first head-pair can start as early as possible
                wvs = []
                for ds in range(ND):
                    wt = w_pool.tile([P, 512], DT_X, tag="w", name="w")
                    dma_in(wt, wvT[ds * P:(ds + 1) * P, :])
                    wvs.append(wt)
                ones_col = w_pool.tile([P, 8, 1], F32, tag="ones_col",
                                       name="ones_col")
                nc.vector.memset(ones_col[:], 1.0)
                for kb in range(NKB):
                    ps = mm_psum.tile([P, TQ], F32, tag="mm", name="mm")
                    for ds in range(ND):
                        nc.tensor.matmul(
                            ps,
                            xt[ds][:, kb * P:(kb + 1) * P],
                            wvs[ds][:],
                            start=(ds == 0), stop=(ds == ND - 1))
                    nc.vector.tensor_copy(
                        vA[kb][:, :, 0:64],
                        ps[:].rearrange("p (h c) -> p h c", c=64))
                    nc.vector.tensor_copy(vA[kb][:, :, 64:65], ones_col[:])

                # q and k, interleaved per head-pair (attention[hp] unblocks
                # after q[hp]+k[hp])
                wqs, wks = [], []
                for w_dram, wts in ((wqT, wqs), (wkT, wks)):
                    for ds in range(ND):
                        wt = w_pool.tile([P, 512], DT_X, tag="w", name="w")
                        dma_in(wt, w_dram[ds * P:(ds + 1) * P, :])
                        wts.append(wt)
                for hp in range(NHP):
                    for wts, outs in ((wqs, qT), (wks, kT)):
                        for tt in range(NQB):
                            ps = mm_psum.tile([P, TQ], F32, tag="mm", name="mm")
                            for ds in range(ND):
                                nc.tensor.matmul(
                                    ps,
                                    wts[ds][:, hp * P:(hp + 1) * P],
                                    xt[ds][:, tt * TQ:(tt + 1) * TQ],
                                    start=(ds == 0), stop=(ds == ND - 1))
                            nc.vector.tensor_copy(
                                outs[hp][:, tt * TQ:(tt + 1) * TQ], ps[:])

            # ---------------- attention ----------------
            ao_pool = ctx.enter_context(tc.tile_pool(name="ao", bufs=NHP))
            aoT = [ao_pool.tile([P, T], DT_X, tag="aoT", name="aoT") for _ in range(NHP)]
            with tc.tile_pool(name="p", bufs=8) as p_pool, \
                 tc.tile_pool(name="r", bufs=8) as r_pool, \
                 tc.tile_pool(name="sps", bufs=2, space="PSUM") as s_psum, \
                 tc.tile_pool(name="ops", bufs=3, space="PSUM") as o_psum:
                ones1 = r_pool.tile([1, 64], F32R, tag="ones1", name="ones1")
                ones1f = r_pool.tile([1, 64], F32, tag="ones1f", name="ones1f")
                nc.vector.memset(ones1f[:], 1.0)
                nc.vector.tensor_copy(ones1[:], ones1f[:])
                for hp in range(NHP):
                    for qb in range(NQB):
                        nkb = 4 * qb + 4   # causal: k tiles with k0 <= q0+511
                        o_ps = [o_psum.tile([P, TQ], F32, tag="o", name="o") for _ in range(2)]
                        for kb in range(nkb):
                            # scores transposed: (k position, q position)
                            s_ps = s_psum.tile([P, 2, TQ], F32, tag="s",
                                               name="s")
                            for j in range(2):
                                nc.tensor.matmul(
                                    s_ps[:, j, :],
                                    kT[hp][j * 64:(j + 1) * 64,
                                           kb * P:(kb + 1) * P],
                                    qT[hp][j * 64:(j + 1) * 64,
                                           qb * TQ:(qb + 1) * TQ],
                                    tile_position=(j * 64, 0))
                            pt = p_pool.tile([P, 2, TQ], DT_P, tag="p", name="p")
                            nc.scalar.activation(pt[:], s_ps[:], AF.Exp)
                            d = qb * TQ - kb * P   # q0 - k0
                            if d <= 0:
                                # diagonal tile: zero the (q < k) entries
                                nc.gpsimd.affine_select(
                                    out=pt[:], in_=pt[:],
                                    pattern=[[0, 2], [1, TQ]],
                                    compare_op=mybir.AluOpType.is_ge,
                                    fill=0.0, base=d, channel_multiplier=-1)
                            for j in range(2):
                                nc.tensor.matmul(
                                    o_ps[j][0:65, :],
                                    vA[kb][:, 2 * hp + j, :],
                                    pt[:, j, :],
                                    start=(kb == 0), stop=(kb == nkb - 1))
                        for j in range(2):
                            # rows 0..63 = unnormalized out.T, row 64 = sum(exp)
                            # sum row -> sbuf (f32r for the broadcast matmul)
                            sc = r_pool.tile([1, TQ], F32R, tag="sc", name="sc")
                            nc.vector.tensor_copy(sc[:], o_ps[j][64:65, :])
                            # broadcast sums across 64 partitions via K=1 matmul
                            # (shares a scores-psum slot briefly)
                            rb = o_psum.tile([64, TQ], F32, tag="rb", name="rb", bufs=1)
                            nc.tensor.matmul(rb[:], ones1[:], sc[:],
                                             start=True, stop=True)
                            # 1/sums on all 64 partitions at once (sums >= 1)
                            R = r_pool.tile([64, TQ], F32, tag="R", name="R")
                            nc.vector.reciprocal_approx_fast(R[:], rb[:])
                            nc.vector.tensor_mul(
                                aoT[hp][j * 64:(j + 1) * 64,
                                        qb * TQ:(qb + 1) * TQ],
                                o_ps[j][0:64, :], R[:])

            # ---------------- output projection (partial) ----------------
            with tc.tile_pool(name="wo", bufs=4) as wo_pool, \
                 tc.tile_pool(name="po", bufs=4) as po_pool, \
                 tc.tile_pool(name="pps", bufs=4, space="PSUM") as p_psum:
                wos = []
                for cs in range(4):
                    wt = wo_pool.tile([P, D], DT_X, tag="wo", name="wo")
                    dma_in(wt, woT[cs * P:(cs + 1) * P, :])
                    wos.append(wt)
                for od in range(ND):
                    for tt in range(NQB):
                        ps = p_psum.tile([P, TQ], F32, tag="pp", name="pp")
                        for cs in range(4):
                            nc.tensor.matmul(
                                ps,
                                wos[cs][:, od * P:(od + 1) * P],
                                aoT[cs][:, tt * TQ:(tt + 1) * TQ],
                                start=(cs == 0), stop=(cs == 3))
                        po = po_pool.tile([P, TQ], F32, tag="po", name="po")
                        nc.vector.tensor_copy(po[:], ps[:])
                        nc.sync.dma_start(
                            pout[od * P:(od + 1) * P, tt * TQ:(tt + 1) * TQ],
                            po[:])

    nc.compile()
    return nc


def _get_compiled(precision=None):
    precision = precision or PRECISION
    if precision not in _COMPILED:
        _COMPILED[precision] = _build(precision)
    return _COMPILED[precision]


def make_in_maps(x, w_q, w_k, w_v, w_o, precision=None):
    precision = precision or PRECISION
    dt = np.float16 if precision == "lp" else np.float32
    xTs = [np.ascontiguousarray(x[b].T).astype(dt) for b in range(B)]
    in_maps = []
    for c in range(8):
        b, half = divmod(c, 2)
        rows = slice(half * 512, (half + 1) * 512)
        in_maps.append({
            "xT": xTs[b],
            "wqT": np.ascontiguousarray((w_q[rows] * 0.125).T).astype(dt),
            "wkT": np.ascontiguousarray(w_k[rows].T).astype(dt),
            "wvT": np.ascontiguousarray(w_v[rows].T).astype(dt),
            "woT": np.ascontiguousarray(w_o[:, rows].T).astype(dt),
        })
    return in_maps


def kernel(x, w_q, w_k, w_v, w_o):
    from concourse.bass_utils import run_bass_kernel_spmd

    x = np.asarray(x, dtype=np.float32)
    w_q = np.asarray(w_q, dtype=np.float32)
    w_k = np.asarray(w_k, dtype=np.float32)
    w_v = np.asarray(w_v, dtype=np.float32)
    w_o = np.asarray(w_o, dtype=np.float32)

    nc = _get_compiled()
    in_maps = make_in_maps(x, w_q, w_k, w_v, w_o)
    res = run_bass_kernel_spmd(nc, in_maps, list(range(8)))

    out = np.empty((B, T, D), dtype=np.float32)
    for b in range(B):
        out[b] = (res.results[2 * b]["poutT"] + res.results[2 * b + 1]["poutT"]).T
    return out



# revision 20
# speedup vs baseline: 1.1635x; 1.1635x over previous
"""Causal self-attention (B=4, T=2048, D=1024, H=16) on 8 Trainium2 cores.

Sharding: core c handles batch b = c // 2 and head-half = c % 2 (8 of the 16
heads). Zero cross-core communication: each core computes q/k/v projections
for its 8 heads, causal flash-style attention, and a partial output
projection against its half of w_o. The host sums the two partial
projections per batch.

v2 schedule (single unified PSUM pool, 8 banks: s=2x2, o=3x1, rb=1):
  - DMA order wv -> x -> wq/wk -> wo; v-projection runs ds-outer in waves of
    6 so the PE starts ~4us into the run instead of waiting for all inputs.
  - qk-projection of head-pair hp+1 is emitted between attention chains of
    hp: pure-tensor filler that lets the scalar engine drain its exp backlog.
  - Diagonal score tiles are restricted to their valid column range
    (columns q_local >= 128*j for the j-th diagonal tile): fewer PE rows,
    ~25% less exp work, and affine_select shrinks to a 128-wide strip.
  - Scores are emitted one k-tile ahead of P@V; normalization of chain i is
    emitted inside chain i+1 so its rb-broadcast matmul never stalls the PE.

Attention math per head (dh=64): scores are computed TRANSPOSED
(k on partitions, q on free dim) so that softmax(score) tiles feed the
P@V matmul directly as the moving operand. Softmax uses no max-subtraction
(scores are O(5), fp32 exp is safe); the denominator is produced by an
extra all-ones column appended to v (M=65 in the P@V matmul); the
normalization multiplies the 64 output rows by 1/sums, with sums broadcast
across partitions via a K=1 ones matmul and inverted by
reciprocal_approx_fast (sums >= 1 always: the k=q diagonal term is
exp(|q|^2/8) >= 1).

PRECISION:
  "f32r" - all matmuls in float32r (TF32-like, ~2.8e-4 absmax error)
  "lp"   - q/k/x/w in fp16, softmax weights/v in bf16 (exp can reach e^30,
           beyond fp16 range), PSUM always fp32. ~1.9e-3 absmax error and
           measurably less DVFS throttling (lower matmul power).
"""
import sys

if "/opt/trn_rl_repo" not in sys.path:
    sys.path.insert(0, "/opt/trn_rl_repo")

import numpy as np

B, T, D, H = 4, 2048, 1024, 16
P, TQ = 128, 512
ND = D // P          # 8  d-slices (contraction tiles for projections)
NHP = 4              # head-pairs per core (8 heads)
NQB = T // TQ        # 4  q blocks
NKB = T // P         # 16 k tiles

PRECISION = "lp"

_COMPILED = {}


def _build(precision):
    import concourse.bacc as bacc
    import concourse.tile as tile
    from concourse import mybir
    from contextlib import ExitStack

    F32 = mybir.dt.float32
    F32R = mybir.dt.float32r
    AF = mybir.ActivationFunctionType

    lp8 = precision == "lp8"
    lp = precision in ("lp", "lp8")
    F8 = mybir.dt.float8e4
    DT_IN = mybir.dt.float16 if lp else F32   # dram input dtype
    DT_X = mybir.dt.float16 if lp else F32R   # x / wq / wk / wv / wo / q / k / ao
    DT_P = mybir.dt.bfloat16 if lp else F32R  # softmax weights and v

    def dma_in(dst, src_ap, eng=None):
        # f32r tiles are fed by bitcasting the f32 dram source; fp16 matches
        (eng or nc.sync).dma_start(
            dst, src_ap.bitcast(DT_X) if not lp else src_ap)

    nc = bacc.Bacc("TRN2", target_bir_lowering=False, debug=False, num_devices=8)

    xT = nc.dram_tensor("xT", [D, T], DT_IN, kind="ExternalInput")
    wqT = nc.dram_tensor("wqT", [D, 512], DT_IN, kind="ExternalInput")
    wkT = nc.dram_tensor("wkT", [D, 512], DT_IN, kind="ExternalInput")
    wvT = nc.dram_tensor("wvT", [D, 512], DT_IN, kind="ExternalInput")
    woT = nc.dram_tensor("woT", [512, D], DT_IN, kind="ExternalInput")
    pout = nc.dram_tensor("poutT", [D, T], F32, kind="ExternalOutput")

    with tile.TileContext(nc) as tc:
        with ExitStack() as ctx:
            q_pool = ctx.enter_context(tc.tile_pool(name="q", bufs=NHP))
            k_pool = ctx.enter_context(tc.tile_pool(name="k", bufs=NHP))
            v_pool = ctx.enter_context(tc.tile_pool(name="v", bufs=NKB))
            xt_pool = ctx.enter_context(tc.tile_pool(name="xt", bufs=ND))
            w_pool = ctx.enter_context(tc.tile_pool(name="w", bufs=24))
            ao_pool = ctx.enter_context(tc.tile_pool(name="ao", bufs=NHP))
            p_pool = ctx.enter_context(tc.tile_pool(name="p", bufs=8))
            r_pool = ctx.enter_context(tc.tile_pool(name="r", bufs=2))
            po_pool = ctx.enter_context(tc.tile_pool(name="po", bufs=4))
            wo_pool = ctx.enter_context(tc.tile_pool(name="wo", bufs=4))
            # one PSUM pool for everything: tags s (2x2 banks), o (3x1),
            # rb (1x1) = 8 banks
            psum = ctx.enter_context(tc.tile_pool(name="ps", bufs=1, space="PSUM"))

            if lp8:
                # q/k live as fp8e4m3 in DoubleRow layout [32, head, ktile,
                # T]: partition p, ktile t holds feature 32*t+p of the head
                # (feature order of q and k permuted identically, which
                # leaves q.k scores unchanged). Filled by a partition-
                # shuffling SBUF-to-SBUF DMA from the fp8 staging tiles.
                q8s = [q_pool.tile([32, 2, 2, T], F8, tag="q8", name="q8")
                       for _ in range(NHP)]
                k8s = [k_pool.tile([32, 2, 2, T], F8, tag="k8", name="k8")
                       for _ in range(NHP)]
                st_pool = ctx.enter_context(tc.tile_pool(name="st", bufs=2))
            else:
                qT = [q_pool.tile([P, T], DT_X, tag="qT", name="qT")
                      for _ in range(NHP)]
                kT = [k_pool.tile([P, T], DT_X, tag="kT", name="kT")
                      for _ in range(NHP)]
            # v, row-major (k-position on partitions), 65th column = 1.0
            vA = [v_pool.tile([P, 8, 65], DT_P, tag="vA", name="vA") for _ in range(NKB)]
            aoT = [ao_pool.tile([P, T], DT_X, tag="aoT", name="aoT") for _ in range(NHP)]

            # ---------------- input DMAs (order = arrival order) ---------
            # sync queue only: (wv, x) pairs interleaved so the ds-th v-proj
            # matmuls can start as soon as pair ds lands, then q/k/o
            # weights. (Issuing weights from the scalar queue stalls the
            # act engine on DMA-ring backpressure - measured slower.)
            wvs, xt = [], []
            for ds in range(ND):
                wt = w_pool.tile([P, 512], DT_X, tag="w", name="w")
                dma_in(wt, wvT[ds * P:(ds + 1) * P, :])
                wvs.append(wt)
                t = xt_pool.tile([P, T], DT_X, tag="xt", name="xt")
                dma_in(t, xT[ds * P:(ds + 1) * P, :])
                xt.append(t)
            wqs, wks = [], []
            for w_dram, wts in ((wqT, wqs), (wkT, wks)):
                for ds in range(ND):
                    wt = w_pool.tile([P, 512], DT_X, tag="w", name="w")
                    dma_in(wt, w_dram[ds * P:(ds + 1) * P, :])
                    wts.append(wt)
            wos = []
            for cs in range(4):
                wt = wo_pool.tile([P, D], DT_X, tag="wo", name="wo")
                dma_in(wt, woT[cs * P:(cs + 1) * P, :])
                wos.append(wt)

            for kb in range(NKB):
                nc.gpsimd.memset(vA[kb][:, :, 64:65], 1.0)

            def psum_tile_1b(i):
                # [128,512] f32 psum tile; cycle tags o,o,o,s,s,rb so up to
                # 6 single-bank accumulations can be in flight for v-proj
                tag, bufs = (("o", 3), ("o", 3), ("o", 3), ("s", 2),
                             ("s", 2), ("rb", 1))[i % 6]
                return psum.tile([P, TQ], F32, tag=tag, bufs=bufs, name="vps")

            # ---------------- v projection (ds-outer waves) --------------
            # wave of 6 kb-groups: each matmul needs only xt[ds]+wvs[ds], so
            # the PE starts as soon as the first x slice lands
            vwaves = [list(range(0, 6)), list(range(6, 12)), list(range(12, 16))]
            for wave in vwaves:
                pss = {kb: psum_tile_1b(i) for i, kb in enumerate(wave)}
                for ds in range(ND):
                    for kb in wave:
                        nc.tensor.matmul(
                            pss[kb],
                            xt[ds][:, kb * P:(kb + 1) * P],
                            wvs[ds][:],
                            start=(ds == 0), stop=(ds == ND - 1))
                for kb in wave:
                    # scalar engine is idle during projections
                    nc.scalar.copy(
                        vA[kb][:, :, 0:64],
                        pss[kb][:].rearrange("p (h c) -> p h c", c=64))

            # ---------------- interleaved qk-proj + attention ------------
            ones1 = r_pool.tile([1, 64], F32R, tag="ones1", name="ones1")
            ones1f = r_pool.tile([1, 64], F32, tag="ones1f", name="ones1f")
            nc.vector.memset(ones1f[:], 1.0)
            nc.vector.tensor_copy(ones1[:], ones1f[:])

            def qk_proj(hp, on_act, after_first_group=None):
                # 8 psum groups ([q|k] x 4 tt), each accumulating 8 ds.
                # after_first_group runs once the first group is emitted:
                # deferred norms flushed there have their sc copies done by
                # the time the PE reaches the rb broadcast matmuls.
                for wts, which in ((wqs, 0), (wks, 1)):
                    st8 = None
                    if lp8:
                        st8 = st_pool.tile([P, T], F8, tag="st8", name="st8")
                    for tt in range(NQB):
                        ps = psum.tile([P, TQ], F32, tag="o", bufs=3,
                                       name="qkps")
                        for ds in range(ND):
                            nc.tensor.matmul(
                                ps,
                                wts[ds][:, hp * P:(hp + 1) * P],
                                xt[ds][:, tt * TQ:(tt + 1) * TQ],
                                start=(ds == 0), stop=(ds == ND - 1))
                        if lp8:
                            nc.vector.tensor_copy(
                                st8[:, tt * TQ:(tt + 1) * TQ], ps[:])
                        elif on_act:
                            nc.scalar.copy(
                                qT[hp][:, tt * TQ:(tt + 1) * TQ]
                                if which == 0 else
                                kT[hp][:, tt * TQ:(tt + 1) * TQ], ps[:])
                        else:
                            nc.vector.tensor_copy(
                                qT[hp][:, tt * TQ:(tt + 1) * TQ]
                                if which == 0 else
                                kT[hp][:, tt * TQ:(tt + 1) * TQ], ps[:])
                        if after_first_group is not None:
                            after_first_group()
                            after_first_group = None
                    if lp8:
                        # partition shuffle into DoubleRow layout: feature
                        # 32t+p of head j sits at staging partition
                        # 64j+32t+p
                        dst = (q8s if which == 0 else k8s)[hp]
                        for j in range(2):
                            for t in range(2):
                                nc.sync.dma_start(
                                    dst[0:32, j, t, :],
                                    st8[64 * j + 32 * t:64 * j + 32 * t + 32, :])

            def emit_norm(hp, qb, o_ps, j):
                # rows 0..63 = unnormalized out.T, row 64 = sum(exp)
                sc = r_pool.tile([1, TQ], F32R, tag="sc", name="sc")
                nc.vector.tensor_copy(sc[:], o_ps[j][64:65, :])
                rb = psum.tile([64, TQ], F32, tag="rb", bufs=1, name="rb")
                nc.tensor.matmul(rb[:], ones1[:], sc[:], start=True, stop=True)
                R = r_pool.tile([64, TQ], F32, tag="R", name="R")
                nc.vector.reciprocal_approx_fast(R[:], rb[:])
                nc.vector.tensor_mul(
                    aoT[hp][j * 64:(j + 1) * 64, qb * TQ:(qb + 1) * TQ],
                    o_ps[j][0:64, :], R[:])

            def emit_S(hp, qb, kb):
                # scores transposed: (k position, q position); diagonal
                # tiles only cover their valid columns c0.. (c0 = 128*j)
                c0 = max(0, (kb - 4 * qb) * P)
                s_ps = psum.tile([P, 2, TQ], F32, tag="s", bufs=2, name="s")
                for j in range(2):
                    if lp8:
                        nc.tensor.matmul(
                            s_ps[:, j, c0:TQ],
                            k8s[hp][0:32, j, :, kb * P:(kb + 1) * P],
                            q8s[hp][0:32, j, :,
                                    qb * TQ + c0:(qb + 1) * TQ],
                            perf_mode=mybir.MatmulPerfMode.DoubleRow)
                    else:
                        nc.tensor.matmul(
                            s_ps[:, j, c0:TQ],
                            kT[hp][j * 64:(j + 1) * 64, kb * P:(kb + 1) * P],
                            qT[hp][j * 64:(j + 1) * 64,
                                   qb * TQ + c0:(qb + 1) * TQ],
                            tile_position=(j * 64, 0))
                return s_ps, c0

            pending = []   # deferred normalizations [(hp, qb, o_ps, j), ...]

            def flush_all():
                while pending:
                    emit_norm(*pending.pop(0))

            def attn_chain(hp, qb):
                # k tiles with k0 <= q0+511; diagonal tiles first so both
                # the start (j=0, full width) and the stop write (last
                # off-diagonal) cover every column of the accumulator.
                # qb=0 has no off-diagonal tiles: its last tile (j=3) runs
                # P@V full width, with a widened affine_select that also
                # zeroes the stale pt columns below the valid range.
                seq = list(range(4 * qb, 4 * qb + 4)) + list(range(4 * qb))
                o_ps = [psum.tile([P, TQ], F32, tag="o", bufs=3, name="o")
                        for _ in range(2)]
                cur = emit_S(hp, qb, seq[0])
                nxt = emit_S(hp, qb, seq[1])
                # previous chain's norms: their rb matmuls land between the
                # score lookahead and PV(0), after their sc copies are done,
                # and their aoT muls retire before any o-slot reuse
                flush_all()
                for i, kb in enumerate(seq):
                    s_ps, c0 = cur
                    cur = nxt
                    nxt = emit_S(hp, qb, seq[i + 2]) if i + 2 < len(seq) else None
                    last = i == len(seq) - 1
                    wide = last and c0 > 0  # qb=0 only: full-width stop
                    pt = p_pool.tile([P, 2, TQ], DT_P, tag="p", name="p")
                    # lp8 leaves q/k unscaled for fp8 range; the 1/8 score
                    # scale is applied here instead of folded into w_q
                    nc.scalar.activation(
                        pt[:, :, c0:TQ], s_ps[:, :, c0:TQ], AF.Exp,
                        scale=0.125 if lp8 else 1.0)
                    if kb >= 4 * qb:
                        # diagonal tile: zero the (q < k) entries; they all
                        # live in the 128 columns at c0 (plus, when wide,
                        # everything below c0, which is stale slot data)
                        lo = 0 if wide else c0
                        nc.gpsimd.affine_select(
                            out=pt[:, :, lo:c0 + P], in_=pt[:, :, lo:c0 + P],
                            pattern=[[0, 2], [1, c0 + P - lo]],
                            compare_op=mybir.AluOpType.is_ge,
                            fill=0.0, base=lo - c0, channel_multiplier=-1)
                    pv0 = 0 if wide else c0
                    for j in range(2):
                        nc.tensor.matmul(
                            o_ps[j][0:65, pv0:TQ],
                            vA[kb][:, 2 * hp + j, :],
                            pt[:, j, pv0:TQ],
                            start=(i == 0), stop=last)
                pending.append((hp, qb, o_ps, 0))
                pending.append((hp, qb, o_ps, 1))

            def out_proj_tt(tt):
                # partial output projection for one 512-column block; all
                # four aoT head-pairs for this block must be normalized
                for od in range(ND):
                    ps = psum.tile([P, TQ], F32, tag="o", bufs=3, name="pp")
                    for cs in range(4):
                        nc.tensor.matmul(
                            ps,
                            wos[cs][:, od * P:(od + 1) * P],
                            aoT[cs][:, tt * TQ:(tt + 1) * TQ],
                            start=(cs == 0), stop=(cs == 3))
                    po = po_pool.tile([P, TQ], F32, tag="po", name="po")
                    nc.vector.tensor_copy(po[:], ps[:])
                    nc.sync.dma_start(
                        pout[od * P:(od + 1) * P, tt * TQ:(tt + 1) * TQ],
                        po[:])

            qk_proj(0, on_act=True)
            for hp in range(NHP):
                for qb in range(NQB):
                    attn_chain(hp, qb)
                    if qb == 0 and hp + 1 < NHP:
                        # qk psums rotate through the o-slots: norms that
                        # could gate slot reuse flush after the first group
                        # (whose slot predates the pending norms)
                        qk_proj(hp + 1, on_act=False,
                                after_first_group=flush_all)
                    if hp == NHP - 1:
                        # last head-pair: column block qb of the output
                        # projection unblocks after this chain's norms
                        flush_all()
                        out_proj_tt(qb)

    nc.compile()
    return nc


def _get_compiled(precision=None):
    precision = precision or PRECISION
    if precision not in _COMPILED:
        _COMPILED[precision] = _build(precision)
    return _COMPILED[precision]


def make_in_maps(x, w_q, w_k, w_v, w_o, precision=None):
    precision = precision or PRECISION
    dt = np.float16 if precision in ("lp", "lp8") else np.float32
    # lp8 applies the 1/8 score scale in the exp activation instead (q/k
    # stay O(1) for fp8e4m3 range)
    qs = 1.0 if precision == "lp8" else 0.125
    xTs = [np.ascontiguousarray(x[b].T).astype(dt) for b in range(B)]
    in_maps = []
    for c in range(8):
        b, half = divmod(c, 2)
        rows = slice(half * 512, (half + 1) * 512)
        in_maps.append({
            "xT": xTs[b],
            "wqT": np.ascontiguousarray((w_q[rows] * qs).T).astype(dt),
            "wkT": np.ascontiguousarray(w_k[rows].T).astype(dt),
            "wvT": np.ascontiguousarray(w_v[rows].T).astype(dt),
            "woT": np.ascontiguousarray(w_o[:, rows].T).astype(dt),
        })
    return in_maps


def kernel(x, w_q, w_k, w_v, w_o):
    from concourse.bass_utils import run_bass_kernel_spmd

    x = np.asarray(x, dtype=np.float32)
    w_q = np.asarray(w_q, dtype=np.float32)
    w_k = np.asarray(w_k, dtype=np.float32)
    w_v = np.asarray(w_v, dtype=np.float32)
    w_o = np.asarray(w_o, dtype=np.float32)

    nc = _get_compiled()
    in_maps = make_in_maps(x, w_q, w_k, w_v, w_o)
    res = run_bass_kernel_spmd(nc, in_maps, list(range(8)))

    out = np.empty((B, T, D), dtype=np.float32)
    for b in range(B):
        out[b] = (res.results[2 * b]["poutT"] + res.results[2 * b + 1]["poutT"]).T
    return out
